# revision 1
# baseline (speedup 1.0000x reference)
"""Differentiable-JPEG Trainium2 kernel (8-core data-parallel, full I/O).

Pipeline per 32-row x 512-col x 3-channel tile (96 packed partitions):
  MM1 (x4):  T = X^T @ A1aug      color transform + H-DCT (+DC bias via
                                  augmented ones row), per 128-col chunk
  MM2:       C = BDt^T @ T        W-DCT (block-diag D^T), one matmul
  quant:     t = tanh(RQ15*C)  (round(d)==0 exactly since |d|<0.5);
             Cq = C + t*HQ
  MM3 (x4):  R = Cq^T @ BD        W-IDCT
  MM4 (x4):  Y = AIaug^T @ R''    H-IDCT + inverse color + 0.5 bias
  clip:      out = min(max(Y,0),1)

Batch dim (32) sharded 4-per-core across 8 NeuronCores; constants
replicated. All matmul biases ride augmented contraction rows (ones rows
kept in persistent SBUF buffers).
"""
import numpy as np

B, C, H, W = 32, 3, 512, 512
NCORES = 8
BPC = B // NCORES           # images per core
G, CCH, XX = 4, 3, 8        # 8-row groups per tile, channels, rows per block
P96 = G * CCH * XX          # 96 packed partitions
NT = H // 32                # 16 h-tiles per image
FREE = NT * W               # 8192 free elements per image buffer
MAGIC = 12582912.0          # 1.5*2^23 fp32 round-to-nearest-even trick

QUALITY = 50.0
_LUM = np.array([[16,11,10,16,24,40,51,61],[12,12,14,19,26,58,60,55],[14,13,16,24,40,57,69,56],[14,17,22,29,51,87,80,62],[18,22,37,56,68,109,103,77],[24,35,55,64,81,104,113,92],[49,64,78,87,103,121,120,101],[72,92,95,98,112,100,103,99]], dtype=np.float32)
_CHR = np.array([[17,18,24,47,99,99,99,99],[18,21,26,66,99,99,99,99],[24,26,56,99,99,99,99,99],[47,66,99,99,99,99,99,99],[99,99,99,99,99,99,99,99],[99,99,99,99,99,99,99,99],[99,99,99,99,99,99,99,99],[99,99,99,99,99,99,99,99]], dtype=np.float32)


def _scaled_qtable(base, qf):
    qf = max(1.0, min(100.0, qf))
    s = 5000.0 / qf if qf < 50 else 200.0 - 2.0 * qf
    return np.maximum(np.floor((base * s + 50.0) / 100.0), 1.0)


def _np_consts():
    qtab = np.stack([_scaled_qtable(_LUM, QUALITY), _scaled_qtable(_CHR, QUALITY),
                     _scaled_qtable(_CHR, QUALITY)]).astype(np.float32)  # [c,u,v]
    u8 = np.arange(8)[:, None]
    x8 = np.arange(8)[None, :]
    cu = np.where(u8 == 0, 1.0 / np.sqrt(2.0), 1.0)
    D = (0.5 * cu * np.cos((2 * x8 + 1) * u8 * np.pi / 16.0)).astype(np.float32)
    MFWD = np.array([[0.299, 0.587, 0.114], [-0.168736, -0.331264, 0.5],
                     [0.5, -0.418688, -0.081312]], np.float32)
    MINV = np.array([[1.0, 0.0, 1.402], [1.0, -0.344136, -0.714136],
                     [1.0, 1.772, 0.0]], np.float32)

    A1 = np.zeros((97, 96), np.float32)
    AI = np.zeros((97, 96), np.float32)
    for g in range(G):
        for c in range(CCH):
            for c2 in range(CCH):
                p0 = c*32 + g*8
                n0 = c2*32 + g*8
                # A1[p=(c,g,xx), n=(c2,g,u)] = MFWD[c2,c] * D[u,xx]
                A1[p0:p0+8, n0:n0+8] = MFWD[c2, c] * D.T
                # AI[k=(c2,g,u), m=(c,g,xx)] = MINV[c,c2] * D[u,xx]
                AI[n0:n0+8, p0:p0+8] = MINV[c, c2] * D
        A1[96, g*8] = -np.sqrt(2.0)     # (c2=Y, u=0): forward -0.5 pixel bias
    AI[96, :] = 0.5                      # +0.5 pixel bias on inverse

    BDt = np.zeros((128, 128), np.float32)
    for a in range(16):
        BDt[8*a:8*a+8, 8*a:8*a+8] = D.T
    BD = np.ascontiguousarray(BDt.T)

    RQ = np.zeros((128, 384), np.float32)
    HQ = np.zeros((128, 384), np.float32)
    v = np.arange(128) % 8
    for j in range(4):
        for c in range(CCH):
            for g in range(G):
                for u in range(XX):
                    col = j*96 + c*32 + g*8 + u
                    RQ[:, col] = 15.0 / qtab[c, u, v]
                    HQ[:, col] = 0.5 * qtab[c, u, v]
    return {"a1": A1, "ai": AI, "bdt": BDt, "bd": BD, "rq": RQ, "hq": HQ}


_CACHE = {}


def _build(work_bufs=3, rsb_n=3, uu_pool=True, dma_split=2, ablate=(), use_f32r="both", scalar_stores=True, mm3_fp16=False):
    import concourse.bacc as bacc
    import concourse.mybir as mybir
    import concourse.tile as tile

    F32 = mybir.dt.float32
    F32R = mybir.dt.float32r
    AOT = mybir.AluOpType
    nc = bacc.Bacc("TRN2", target_bir_lowering=False, debug=False)

    x = nc.dram_tensor("x", [BPC, C, H, W], F32, kind="ExternalInput")
    out = nc.dram_tensor("out", [BPC, C, H, W], F32, kind="ExternalOutput")
    if use_f32r == "both" or use_f32r is True:
        _r = {"bdt", "ai"}
    elif use_f32r == "mm4":
        _r = {"ai"}
    else:
        _r = set()
    F16 = mybir.dt.float16
    _h = {"bd"} if mm3_fp16 else set()
    cd = {k: nc.dram_tensor(
              k, list(vv.shape),
              F32R if k in _r else (F16 if k in _h else F32),
              kind="ExternalInput")
          for k, vv in _np_consts().items()}

    # per-(image, channel) packed APs: partitions (g xx), free (t, w)
    xin_src = x.ap().rearrange("b c (t g xx) w -> b c (g xx) t w", t=NT, g=G, xx=XX)
    out_dst = out.ap().rearrange("b c (t g xx) w -> b c (g xx) t w", t=NT, g=G, xx=XX)

    with tile.TileContext(nc) as tc:
        # persistent SBUF state
        csb = {k: nc.alloc_sbuf_tensor(
                   f"c_{k}", list(v.shape),
                   F32R if k in _r else (F16 if k in _h else F32))
               for k, v in _np_consts().items()}
        xin = [nc.alloc_sbuf_tensor(f"xin{i}", [97, FREE], F32) for i in range(2)]
        rout = [nc.alloc_sbuf_tensor(f"rout{i}", [P96, FREE], F32) for i in range(2)]
        rsb = [nc.alloc_sbuf_tensor(f"rsb{i}", [97, W], F32R if "ai" in _r else F32) for i in range(rsb_n)]
        zbias = nc.alloc_sbuf_tensor("zbias", [128, 1], F32)

        for k, t in csb.items():
            nc.sync.dma_start(out=t.ap(), in_=cd[k].ap())
        nc.vector.memset(zbias.ap(), 0.0)
        for i in range(2):
            nc.vector.memset(xin[i].ap()[96:97, :], 1.0)
        for i in range(rsb_n):
            nc.vector.memset(rsb[i].ap()[96:97, :].bitcast(F32), 1.0)

        a1, ai = csb["a1"].ap(), csb["ai"].ap()
        bdt, bd = csb["bdt"].ap(), csb["bd"].ap()
        rq, hq = csb["rq"].ap(), csb["hq"].ap()
        zb = zbias.ap()

        with (
            tc.tile_pool(name="psT", bufs=3, space="PSUM") as psT,
            tc.tile_pool(name="psC", bufs=2, space="PSUM") as psC,
            tc.tile_pool(name="psR", bufs=2, space="PSUM") as psR,
            tc.tile_pool(name="psY", bufs=1, space="PSUM") as psY,
            tc.tile_pool(name="work", bufs=work_bufs) as work,
        ):
            tchunk = NT // dma_split

            def load_image(b):
                if "dma" in ablate:
                    return
                xv = xin[b % 2].ap()
                for c in range(CCH):
                    for s0 in range(dma_split):
                        nc.sync.dma_start(
                            out=xv[c*32:(c+1)*32,
                                   s0*tchunk*W:(s0+1)*tchunk*W].rearrange(
                                "p (t w) -> p t w", t=tchunk),
                            in_=xin_src[b, c, :, s0*tchunk:(s0+1)*tchunk])

            def store_half(b, s0):
                if "dma" in ablate:
                    return
                ov = rout[b % 2].ap()
                for c in range(CCH):
                    (nc.scalar if scalar_stores else nc.sync).dma_start(
                        out=out_dst[b, c, :, s0*tchunk:(s0+1)*tchunk],
                        in_=ov[c*32:(c+1)*32,
                               s0*tchunk*W:(s0+1)*tchunk*W].rearrange(
                            "p (t w) -> p t w", t=tchunk))

            # 5-stage software pipeline over the 64 (image, h-tile) items:
            #   it j:   MM1(j)x4 -> T_ps ; T-copy -> t_sb
            #   it j+1: MM2(j) -> C_ps ; d15(j) ; tanh(j)
            #   it j+2: uu(j) ; cq(j)      (C_ps lives 2 iters -> psC bufs 3)
            #   it j+3: MM3(j)x4 -> R_ps ; R-copy -> rsb
            #   it j+4: MM4(j) ; clip(j)
            # Emission order keeps every engine queue dependency-clean so no
            # FIFO stream ever stalls on a same-iteration producer.
            items = [(b, t) for b in range(BPC) for t in range(NT)]
            NI = len(items)
            st = {}   # per-item live tiles

            load_image(0)
            if BPC > 1:
                load_image(1)

            for i in range(NI + 4):
                # PE ops first: all deps are >= 1 iteration old.
                if i < NI:
                    b, t = items[i]
                    xv = xin[b % 2].ap()
                    base = t * W
                    T_ps = psT.tile([128, 384], F32)
                    for j in range(4):
                        nc.tensor.matmul(
                            T_ps[:, 96*j:96*j+96],
                            xv[0:97, base+128*j:base+128*j+128],
                            a1, start=True, stop=True)
                    st[i] = {"T_ps": T_ps, "b": b, "t": t}
                if i - 1 >= 0 and i - 1 < NI:
                    e = st[i - 1]
                    C_ps = psC.tile([128, 384], F32)
                    nc.tensor.matmul(C_ps[:, :], bdt, e["t_sb"],
                                     start=True, stop=True)
                    e["C_ps"] = C_ps
                if i - 3 >= 0 and i - 3 < NI:
                    e = st[i - 3]
                    R_ps = psR.tile([P96, W], F32)
                    for j in range(4):
                        nc.tensor.matmul(
                            R_ps[:, 128*j:128*j+128],
                            e["cq"][:, 96*j:96*j+96],
                            bd, start=True, stop=True)
                    e["R_ps"] = R_ps
                if i - 4 >= 0 and i - 4 < NI:
                    e = st[i - 4]
                    Y_ps = psY.tile([P96, W], F32)
                    nc.tensor.matmul(Y_ps[:, :], ai, e["rv"][0:97, :],
                                     start=True, stop=True)
                    e["Y_ps"] = Y_ps

                # ACT ops
                if i < NI:
                    e = st[i]
                    t_sb = work.tile([128, 384], F32R if "bdt" in _r else F32, tag="t_sb")
                    nc.scalar.copy(t_sb, e["T_ps"][:, :])
                    e["t_sb"] = t_sb
                if i - 1 >= 0 and i - 1 < NI and "quant" not in ablate:
                    e = st[i - 1]
                    # |d| = |C|/q <= 4.0/10 < 0.5 always => round(d) == 0,
                    # so tanh(15(d-round(d))) == tanh(RQ15*C) exactly.
                    d15 = work.tile([128, 384], F32, tag="d15")
                    nc.vector.tensor_tensor(d15, e["C_ps"][:, :], rq, AOT.mult)
                    tt = work.tile([128, 384], F32, tag="tt")
                    nc.scalar.activation(tt, d15,
                                         mybir.ActivationFunctionType.Tanh,
                                         bias=zb, scale=1.0)
                    e["tt"] = tt
                if i - 3 >= 0 and i - 3 < NI:
                    e = st[i - 3]
                    rv = rsb[(i - 3) % rsb_n].ap()
                    nc.scalar.copy(rv[0:P96, :], e["R_ps"][:, :])
                    e["rv"] = rv

                # POOL + DVE quant tail
                if i - 2 >= 0 and i - 2 < NI:
                    e = st[i - 2]
                    if "quant" in ablate:
                        e["cq"] = e["t_sb"].bitcast(F32)
                    else:
                        uu = work.tile([128, 384], F32, tag="uu")
                        (nc.gpsimd if uu_pool else nc.vector).tensor_tensor(
                            uu, e["tt"], hq, AOT.mult)
                        cq = work.tile([128, 384], F16 if mm3_fp16 else F32,
                                       tag="cq")
                        nc.vector.tensor_tensor(cq, e["C_ps"][:, :], uu, AOT.add)
                        e["cq"] = cq
                if i - 4 >= 0 and i - 4 < NI:
                    e = st[i - 4]
                    ov = rout[e["b"] % 2].ap()
                    if "clip" not in ablate:
                        nc.vector.tensor_scalar(
                            ov[:, e["t"]*W:(e["t"]+1)*W], e["Y_ps"][:, :], 0.0, 1.0,
                            AOT.max, AOT.min)
                    if e["t"] == NT - 1:
                        for s0 in range(dma_split):
                            store_half(e["b"], s0)
                    del st[i - 4]

                # prefetch: at the first tile of image b, all of image
                # b-1's MM1s are already emitted, so overwriting
                # xin[(b+1)%2] for image b+1 is safe in program order.
                if i < NI:
                    b, t = items[i]
                    if t == 0 and b >= 1 and b + 1 < BPC:
                        load_image(b + 1)
    nc.compile()
    return nc


def _get_nc(**kw):
    key = tuple(sorted(kw.items()))
    if key not in _CACHE:
        _CACHE[key] = _build(**kw)
    return _CACHE[key]


def kernel(x, trace=False, **kw):
    from concourse import bass_utils
    nc = _get_nc(**kw)
    consts = _np_consts()
    if kw.get("mm3_fp16"):
        consts["bd"] = consts["bd"].astype(np.float16)
    x = np.ascontiguousarray(np.asarray(x), dtype=np.float32)
    in_maps = []
    for i in range(NCORES):
        m = {"x": x[i*BPC:(i+1)*BPC]}
        m.update(consts)
        in_maps.append(m)
    try:
        res = bass_utils.run_bass_kernel_spmd(
            nc, in_maps, core_ids=list(range(NCORES)), trace=trace)
    except Exception:
        if not trace:
            raise
        res = bass_utils.run_bass_kernel_spmd(
            nc, in_maps, core_ids=list(range(NCORES)), trace=False)
    _CACHE["last"] = res
    return np.concatenate([r["out"] for r in res.results], axis=0)


def last_exec_time_ns():
    res = _CACHE.get("last")
    return None if res is None else res.exec_time_ns



# revision 22
# speedup vs baseline: 1.3447x; 1.3447x over previous
"""Differentiable-JPEG Trainium2 kernel (8-core data-parallel, full I/O).

Per 32-row x 512-col x 3-channel tile (96 packed partitions; coefficient
columns ordered m = u*12 + c2*4 + g so each (u, luma/chroma) group is a
contiguous slice):
  conv:      xin16 = fp16(x)      POOL 2-item batch (SBUF->SBUF)
  MM1 (x4):  T = X16^T @ A1aug    color transform + V-DCT (+DC bias via
                                  augmented ones row), per 128-col chunk
  MM2 (x16): d15 = BQ[u,cls]^T @ T   H-DCT with 15/q folded into 16
                                  pre-scaled block-diag constants (q is
                                  constant within each (u, cls, v) group;
                                  v-dependence rides the constant's rows).
                                  Writes the tanh argument straight to
                                  PSUM -- no separate d15 pass.
  quant:     tt = tanh(d15) (ACT, PSUM->SBUF fp16); uu = tt*HQ (DVE fp16 2x)
             (|d| = |C|/q <= 4.0/10 < 0.5 always => round(d) == 0, so
              tanh(15(d-round(d))) == tanh(RQ15*C) exactly)
  MM3 (x8):  R = uu^T @ BD + T^T @ I    H-IDCT of uu accumulated with the
                                  identity-matmul of t_sb (R = IDCT_H(C+uu)
                                  = T + IDCT_H(uu)); kills the cq-add op.
  MM4:       Y = AIaug^T @ R''    V-IDCT + inverse color + 0.5 bias
  clip:      out = min(max(Y,0),1) (DVE)

All matmuls run both operands fp16 (1 cyc/row on the PE regardless of
free size; walrus requires matching dtypes when f32/f32r is involved and
bans GPSIMD from PSUM, which shapes the engine assignment below).

Software pipeline: every cross-engine input is >= 1 iteration old.
Stage of item j at loop iteration i:
  j (odd)  : conv pair (POOL)         j + 6  : uu (DVE)
  j + 2    : MM1 x4 -> T_ps           j + 7  : MM3 x8 -> R_ps
  j + 3    : t_sb copy (ACT/DVE split)j + 8  : rsb copy (ACT)
  j + 4    : MM2 x16 -> d15_ps        j + 9  : MM4 -> Y_ps
  j + 5    : tanh (ACT)               j + 10 : clip (DVE) + store

Batch dim (32) sharded 4-per-core across 8 NeuronCores; constants
replicated.
"""
import numpy as np

B, C, H, W = 32, 3, 512, 512
NCORES = 8
BPC = B // NCORES           # images per core
G, CCH, XX = 4, 3, 8        # 8-row groups per tile, channels, rows per block
P96 = G * CCH * XX          # 96 packed partitions
NT = H // 32                # 16 h-tiles per image
FREE = NT * W               # 8192 free elements per image buffer

QUALITY = 50.0
_LUM = np.array([[16,11,10,16,24,40,51,61],[12,12,14,19,26,58,60,55],[14,13,16,24,40,57,69,56],[14,17,22,29,51,87,80,62],[18,22,37,56,68,109,103,77],[24,35,55,64,81,104,113,92],[49,64,78,87,103,121,120,101],[72,92,95,98,112,100,103,99]], dtype=np.float32)
_CHR = np.array([[17,18,24,47,99,99,99,99],[18,21,26,66,99,99,99,99],[24,26,56,99,99,99,99,99],[47,66,99,99,99,99,99,99],[99,99,99,99,99,99,99,99],[99,99,99,99,99,99,99,99],[99,99,99,99,99,99,99,99],[99,99,99,99,99,99,99,99]], dtype=np.float32)


def _scaled_qtable(base, qf):
    qf = max(1.0, min(100.0, qf))
    s = 5000.0 / qf if qf < 50 else 200.0 - 2.0 * qf
    return np.maximum(np.floor((base * s + 50.0) / 100.0), 1.0)


def _np_consts():
    qtab = np.stack([_scaled_qtable(_LUM, QUALITY),
                     _scaled_qtable(_CHR, QUALITY)]).astype(np.float32)  # [cls,u,v]
    u8 = np.arange(8)[:, None]
    x8 = np.arange(8)[None, :]
    cu = np.where(u8 == 0, 1.0 / np.sqrt(2.0), 1.0)
    D = (0.5 * cu * np.cos((2 * x8 + 1) * u8 * np.pi / 16.0)).astype(np.float32)
    MFWD = np.array([[0.299, 0.587, 0.114], [-0.168736, -0.331264, 0.5],
                     [0.5, -0.418688, -0.081312]], np.float32)
    MINV = np.array([[1.0, 0.0, 1.402], [1.0, -0.344136, -0.714136],
                     [1.0, 1.772, 0.0]], np.float32)

    def m_of(u, c2, g):
        return u * 12 + c2 * 4 + g

    A1 = np.zeros((97, 96), np.float32)   # cols m = (u, c2, g)
    AI = np.zeros((97, 96), np.float32)   # rows m, cols (c, g, xx)
    for g in range(G):
        for c in range(CCH):
            for c2 in range(CCH):
                p0 = c*32 + g*8
                for u in range(XX):
                    A1[p0:p0+8, m_of(u, c2, g)] = MFWD[c2, c] * D[u, :]
                    AI[m_of(u, c2, g), p0:p0+8] = MINV[c, c2] * D[u, :]
        A1[96, m_of(0, 0, g)] = -np.sqrt(2.0)   # forward -0.5 pixel bias
    AI[96, :] = 0.5                              # +0.5 pixel bias on inverse

    # BQ: 16 block-diag H-DCT stationaries with 15/q folded in; k = u*2+cls
    BQ = np.zeros((128, 16 * 128), np.float32)
    BD = np.zeros((128, 128), np.float32)
    for wb in range(16):
        for v in range(8):
            for w8 in range(8):
                BD[wb*8+v, wb*8+w8] = D[v, w8]
                for u in range(8):
                    for cls in range(2):
                        BQ[wb*8+w8, (u*2+cls)*128 + wb*8+v] = (
                            D[v, w8] * 15.0 / qtab[cls, u, v])
    I128 = np.eye(128, dtype=np.float32)

    HQ = np.zeros((128, 384), np.float32)
    v = np.arange(128) % 8
    for j in range(4):
        for u in range(XX):
            for c2 in range(CCH):
                for g in range(G):
                    col = j*96 + m_of(u, c2, g)
                    HQ[:, col] = 0.5 * qtab[min(c2, 1), u, v]
    ONES = np.ones((1, FREE), np.float32)
    return {"a1": A1, "ai": AI, "bq": BQ, "bd": BD, "i128": I128,
            "hq": HQ, "ones": ONES}


_CACHE = {}

_FP16_CONSTS = {"a1", "ai", "bq", "bd", "i128", "hq", "ones"}


def _build(tsb_dve=300, clip_dve=512, conv_eng="gpsimd", rsb_eng="act",
           dma_split=2, store_eng="scalar", store_at_half=True,
           tsb_n=8, rsb_n=4, tu_n=3, ablate=()):
    import concourse.bacc as bacc
    import concourse.mybir as mybir
    import concourse.tile as tile

    F32 = mybir.dt.float32
    F16 = mybir.dt.float16
    AOT = mybir.AluOpType
    ACTF = mybir.ActivationFunctionType
    nc = bacc.Bacc("TRN2", target_bir_lowering=False, debug=False)

    x = nc.dram_tensor("x", [BPC, C, H, W], F32, kind="ExternalInput")
    out = nc.dram_tensor("out", [BPC, C, H, W], F32, kind="ExternalOutput")

    def cdt(k):
        return F16 if k in _FP16_CONSTS else F32

    cd = {k: nc.dram_tensor(k, list(vv.shape), cdt(k), kind="ExternalInput")
          for k, vv in _np_consts().items()}

    xin_src = x.ap().rearrange("b c (t g xx) w -> b c (g xx) t w", t=NT, g=G, xx=XX)
    out_dst = out.ap().rearrange("b c (t g xx) w -> b c (g xx) t w", t=NT, g=G, xx=XX)

    with tile.TileContext(nc) as tc:
        csb = {k: nc.alloc_sbuf_tensor(f"c_{k}", list(v.shape), cdt(k))
               for k, v in _np_consts().items() if k != "ones"}
        xin = [nc.alloc_sbuf_tensor(f"xin{i}", [P96, FREE], F32) for i in range(2)]
        x16 = [nc.alloc_sbuf_tensor(f"x16_{i}", [97, FREE], F16) for i in range(2)]
        rout = [nc.alloc_sbuf_tensor(f"rout{i}", [P96, FREE], F32) for i in range(2)]
        rsb = [nc.alloc_sbuf_tensor(f"rsb{i}", [97, W], F16) for i in range(rsb_n)]
        tsb = [nc.alloc_sbuf_tensor(f"tsb{i}", [128, 384], F16) for i in range(tsb_n)]
        ttb = [nc.alloc_sbuf_tensor(f"ttb{i}", [128, 384], F16) for i in range(tu_n)]
        uub = [nc.alloc_sbuf_tensor(f"uub{i}", [128, 384], F16) for i in range(tu_n)]
        zbias = nc.alloc_sbuf_tensor("zbias", [128, 1], F32)
        d15ps = [nc.alloc_psum_tensor(f"d15ps{i}", [128, 384], F32)
                 for i in range(2)]

        for k, t in csb.items():
            nc.sync.dma_start(out=t.ap(), in_=cd[k].ap())
        ones = cd["ones"].ap()
        nc.vector.memset(zbias.ap(), 0.0)
        for i in range(2):
            nc.sync.dma_start(out=x16[i].ap()[96:97, :], in_=ones)
        for i in range(rsb_n):
            nc.sync.dma_start(out=rsb[i].ap()[96:97, :], in_=ones[0:1, 0:W])

        a16, ai16 = csb["a1"].ap(), csb["ai"].ap()
        bq16, bd16 = csb["bq"].ap(), csb["bd"].ap()
        i128 = csb["i128"].ap()
        hq16 = csb["hq"].ap()
        zb = zbias.ap()

        with (
            tc.tile_pool(name="psT", bufs=2, space="PSUM") as psT,
            tc.tile_pool(name="psR", bufs=2, space="PSUM") as psR,
            tc.tile_pool(name="psY", bufs=2, space="PSUM") as psY,
        ):
            tchunk = NT // dma_split

            def load_image(b):
                if "dma" in ablate:
                    return
                xv = xin[b % 2].ap()
                for c in range(CCH):
                    for s0 in range(dma_split):
                        nc.sync.dma_start(
                            out=xv[c*32:(c+1)*32,
                                   s0*tchunk*W:(s0+1)*tchunk*W].rearrange(
                                "p (t w) -> p t w", t=tchunk),
                            in_=xin_src[b, c, :, s0*tchunk:(s0+1)*tchunk])

            def store_half(b, s0):
                if "dma" in ablate:
                    return
                ov = rout[b % 2].ap()
                eng = {"gpsimd": nc.gpsimd, "scalar": nc.scalar,
                       "sync": nc.sync, "vector": nc.vector}[store_eng]
                for c in range(CCH):
                    eng.dma_start(
                        out=out_dst[b, c, :, s0*tchunk:(s0+1)*tchunk],
                        in_=ov[c*32:(c+1)*32,
                               s0*tchunk*W:(s0+1)*tchunk*W].rearrange(
                            "p (t w) -> p t w", t=tchunk))

            items = [(b, t) for b in range(BPC) for t in range(NT)]
            NI = len(items)
            st = {}

            load_image(0)
            if BPC > 1:
                load_image(1)

            CONV_E = {"vector": nc.vector, "scalar": nc.scalar,
                      "gpsimd": nc.gpsimd}[conv_eng]

            for i in range(NI + 11):
                # ---- PE, oldest stage first
                if 0 <= i - 9 < NI:
                    e = st[i - 9]
                    Y_ps = psY.tile([P96, W], F32)
                    nc.tensor.matmul(Y_ps[:, :], ai16,
                                     rsb[(i - 9) % rsb_n].ap()[0:97, :],
                                     start=True, stop=True)
                    e["Y_ps"] = Y_ps
                if 0 <= i - 7 < NI:
                    n = i - 7
                    e = st[n]
                    R_ps = psR.tile([P96, W], F32)
                    uv = uub[n % tu_n].ap()
                    tv = tsb[n % tsb_n].ap()
                    for j in range(4):
                        nc.tensor.matmul(
                            R_ps[:, 128*j:128*j+128],
                            uv[:, 96*j:96*j+96],
                            bd16, start=True, stop=False)
                        nc.tensor.matmul(
                            R_ps[:, 128*j:128*j+128],
                            tv[:, 96*j:96*j+96],
                            i128, start=False, stop=True)
                    e["R_ps"] = R_ps
                if 0 <= i - 4 < NI:
                    n = i - 4
                    # 16 sub-matmuls: moving cols (j, m-range) per (u, cls)
                    tr = tsb[n % tsb_n].ap().rearrange(
                        "p (j u m) -> p j u m", j=4, u=8, m=12)
                    dr = d15ps[n % 2].ap().rearrange(
                        "p (j u m) -> p j u m", j=4, u=8, m=12)
                    for u in range(8):
                        for cls in range(2):
                            m0, m1 = (0, 4) if cls == 0 else (4, 12)
                            nc.tensor.matmul(
                                dr[:, :, u, m0:m1],
                                bq16[:, (u*2+cls)*128:(u*2+cls)*128+128],
                                tr[:, :, u, m0:m1],
                                start=True, stop=True)
                if 0 <= i - 2 < NI:
                    n = i - 2
                    b, t = items[n]
                    xv16 = x16[b % 2].ap()
                    base = t * W
                    T_ps = psT.tile([128, 384], F32)
                    for j in range(4):
                        nc.tensor.matmul(
                            T_ps[:, 96*j:96*j+96],
                            xv16[0:97, base+128*j:base+128*j+128],
                            a16, start=True, stop=True)
                    st[n] = {"T_ps": T_ps, "b": b, "t": t}

                # ---- ACT: rsb copy (i-8), tanh (i-5), t_sb tail (i-3)
                if 0 <= i - 8 < NI and "rsb" not in ablate:
                    n = i - 8
                    dst = rsb[n % rsb_n].ap()[0:P96, :]
                    if rsb_eng == "act":
                        nc.scalar.copy(dst, st[n]["R_ps"][:, :])
                    else:
                        nc.vector.tensor_scalar(dst, st[n]["R_ps"][:, :],
                                                1.0, None, AOT.mult)
                if 0 <= i - 5 < NI and "tanh" not in ablate:
                    n = i - 5
                    nc.scalar.activation(ttb[n % tu_n].ap(),
                                         d15ps[n % 2].ap(),
                                         ACTF.Tanh, bias=zb, scale=1.0)
                if 0 <= i - 3 < NI and tsb_dve < 384 and "tsb" not in ablate:
                    n = i - 3
                    nc.scalar.copy(tsb[n % tsb_n].ap()[:, tsb_dve:384],
                                   st[n]["T_ps"][:, tsb_dve:384])

                # ---- DVE: clip (i-10), uu (i-6), t_sb head (i-3)
                if 0 <= i - 10 < NI:
                    e = st[i - 10]
                    ov = rout[e["b"] % 2].ap()
                    t0 = e["t"] * W
                    if "clip" not in ablate:
                        nc.vector.tensor_scalar(
                            ov[:, t0:t0+W], e["Y_ps"][:, :], 0.0, 1.0,
                            AOT.max, AOT.min)
                    if store_at_half:
                        if (e["t"] + 1) % (NT // dma_split) == 0:
                            store_half(e["b"], (e["t"] + 1) // (NT // dma_split) - 1)
                    elif e["t"] == NT - 1:
                        for s0 in range(dma_split):
                            store_half(e["b"], s0)
                    del st[i - 10]
                if 0 <= i - 6 < NI and "uu" not in ablate:
                    n = i - 6
                    nc.vector.tensor_tensor(uub[n % tu_n].ap(),
                                            ttb[n % tu_n].ap(),
                                            hq16, AOT.mult)
                if 0 <= i - 3 < NI and tsb_dve > 0 and "tsb" not in ablate:
                    n = i - 3
                    nc.vector.tensor_scalar(
                        tsb[n % tsb_n].ap()[:, 0:tsb_dve],
                        st[n]["T_ps"][:, 0:tsb_dve], 1.0, None, AOT.mult)

                # ---- POOL: conv pair (i odd)
                if 0 <= i < NI and i % 2 == 1 and "conv" not in ablate:
                    b, t = items[i]
                    base = (t - 1) * W
                    CONV_E.tensor_scalar(
                        x16[b % 2].ap()[0:P96, base:base+2*W],
                        xin[b % 2].ap()[0:P96, base:base+2*W],
                        1.0, None, AOT.mult)

                # prefetch next image's input
                if 0 <= i - 2 < NI:
                    b, t = items[i - 2]
                    if t == 0 and b >= 1 and b + 1 < BPC:
                        load_image(b + 1)
    nc.compile()
    return nc


def _get_nc(**kw):
    key = tuple(sorted(kw.items()))
    if key not in _CACHE:
        _CACHE[key] = _build(**kw)
    return _CACHE[key]


def kernel(x, trace=False, **kw):
    from concourse import bass_utils
    nc = _get_nc(**kw)
    consts = _np_consts()
    for k in _FP16_CONSTS:
        consts[k] = consts[k].astype(np.float16)
    x = np.ascontiguousarray(np.asarray(x), dtype=np.float32)
    in_maps = []
    for i in range(NCORES):
        m = {"x": x[i*BPC:(i+1)*BPC]}
        m.update(consts)
        in_maps.append(m)
    try:
        res = bass_utils.run_bass_kernel_spmd(
            nc, in_maps, core_ids=list(range(NCORES)), trace=trace)
    except Exception:
        if not trace:
            raise
        res = bass_utils.run_bass_kernel_spmd(
            nc, in_maps, core_ids=list(range(NCORES)), trace=False)
    _CACHE["last"] = res
    return np.concatenate([r["out"] for r in res.results], axis=0)


def last_exec_time_ns():
    res = _CACHE.get("last")
    return None if res is None else res.exec_time_ns
